# revision 4
# baseline (speedup 1.0000x reference)
"""CNN-LSTM-CRF kernel for 8 Trainium2 NeuronCores.

Sharding: 8 cores = 4 batch quarters x (fwd, bwd) LSTM direction.
Core 2q runs the forward LSTM for batch quarter q, core 2q+1 runs the
backward LSTM (fed l-reversed tokens + l-reversed conv weights, so the
program is identical on every core - only the data differs).  Each core:
  embedding gather -> conv (PE, e-major) -> fused conv-proj + recurrent
  LSTM steps (PE h-stationary, weights streamed) -> inline classifier
  partial emissions -> pair AllReduce -> full Viterbi fwd scan +
  backtrace (redundant within a pair; host reads the even cores).
"""

import numpy as np

_CACHE = {}

E, H, V, T = 300, 512, 30001, 31
G = 4 * H  # 2048 gate width
EA = E + 1  # contract dim with bias row


def _build(L, BQ):
    import concourse.bass as bass
    import concourse.tile as tile
    from concourse import bacc, mybir
    from concourse.masks import make_identity

    dt = mybir.dt
    f32 = dt.float32
    AF = mybir.ActivationFunctionType
    OP = mybir.AluOpType

    # e-chunks for the LSTM input projection (incl. bias/ones row)
    KC_E = [(0, 128), (128, 128), (256, EA - 256)]  # rows 128,128,45
    KC_CONV = [(0, 128), (128, 128), (256, E - 256)]  # rows 128,128,44
    NKH = H // 128  # 4 h chunks
    # lin chunks (conv contraction over sequence positions)
    LIN = [(0, min(128, L))] + ([(128, L - 128)] if L > 128 else [])
    # l-chunking of the conv_e ring buffer
    LCH = 20 if L % 20 == 0 else L
    NCH = L // LCH

    nc = bacc.Bacc("TRN2", target_bir_lowering=False, debug=False, num_devices=8)

    t_tokT = nc.dram_tensor("tokT", [L, BQ], dt.int32, kind="ExternalInput")
    t_table = nc.dram_tensor("table", [V, E], f32, kind="ExternalInput")
    t_cwT = nc.dram_tensor("cwT", [L, 3 * L], f32, kind="ExternalInput")
    t_cb = nc.dram_tensor("cb", [1, L], f32, kind="ExternalInput")
    t_wih = nc.dram_tensor("wih", [3, 128, G], f32, kind="ExternalInput")
    t_whh = nc.dram_tensor("whh", [NKH, 128, G], f32, kind="ExternalInput")
    t_cls = nc.dram_tensor("cls", [2, NKH, 128, T], f32, kind="ExternalInput")
    t_clsb = nc.dram_tensor("clsb", [1, T], f32, kind="ExternalInput")
    t_st = nc.dram_tensor("st", [1, T], f32, kind="ExternalInput")
    t_et = nc.dram_tensor("et", [1, T], f32, kind="ExternalInput")
    t_trT = nc.dram_tensor("trT", [1, T * T], f32, kind="ExternalInput")
    t_tr31 = nc.dram_tensor("tr31", [T, T], f32, kind="ExternalInput")

    o_tags = nc.dram_tensor("o_tags", [BQ, L], dt.int32, kind="ExternalOutput")
    o_score = nc.dram_tensor("o_score", [BQ, 1], f32, kind="ExternalOutput")

    d_spill = nc.dram_tensor("spill", [max(NCH - 1, 1), BQ, 3, 128, LCH], f32,
                             kind="Internal")
    d_ccin = nc.dram_tensor("ccin", [BQ, L * T], f32, kind="Internal")
    d_ccout = nc.dram_tensor("ccout", [BQ, L * T], f32, kind="Internal")

    em_sb = nc.alloc_sbuf_tensor("em_sb", [BQ, L * T], f32).ap()
    sc_sb = nc.alloc_sbuf_tensor("sc_sb", [BQ, L * T], f32).ap()
    tags_sb = nc.alloc_sbuf_tensor("tags_sb", [BQ, L], f32).ap()

    with tile.TileContext(nc) as tc:
        with tc.tile_pool(name="wp", bufs=1) as wp, \
             tc.tile_pool(name="ceqp", bufs=2) as ceqp, \
             tc.tile_pool(name="pst", bufs=2, space="PSUM") as pst:

            # ---------- persistent loads ----------
            wih = wp.tile([128, 3 * G], f32, tag="wih")
            for c in range(3):
                nc.sync.dma_start(wih[:, c * G:(c + 1) * G], t_wih[c])
            whh = wp.tile([128, NKH * G], f32, tag="whh")
            for c in range(NKH):
                nc.sync.dma_start(whh[:, c * G:(c + 1) * G], t_whh[c])
            cls = wp.tile([128, 2 * NKH * T], f32, tag="cls")
            for ab in range(2):
                for c in range(NKH):
                    j = (ab * NKH + c) * T
                    nc.sync.dma_start(cls[:, j:j + T], t_cls[ab, c])
            cwT = []
            for i, (ls, lr) in enumerate(LIN):
                cw_t = wp.tile([lr, 3 * L], f32, tag=f"cwT{i}", name=f"cwT{i}")
                nc.sync.dma_start(cw_t[:], t_cwT[ls:ls + lr, :])
                cwT.append(cw_t)
            cb_rep = wp.tile([128, L], f32, tag="cb")
            nc.sync.dma_start(cb_rep[:], t_cb[:].to_broadcast([128, L]))
            clsb_rep = wp.tile([BQ, T], f32, tag="clsb")
            nc.sync.dma_start(clsb_rep[:], t_clsb[:].to_broadcast([BQ, T]))
            st_rep = wp.tile([BQ, T], f32, tag="st")
            nc.sync.dma_start(st_rep[:], t_st[:].to_broadcast([BQ, T]))
            et_rep = wp.tile([BQ, T], f32, tag="et")
            nc.sync.dma_start(et_rep[:], t_et[:].to_broadcast([BQ, T]))
            trT_rep = wp.tile([BQ, T * T], f32, tag="trT")
            nc.sync.dma_start(trT_rep[:], t_trT[:].to_broadcast([BQ, T * T]))
            tr31 = wp.tile([T, T], f32, tag="tr31")
            nc.sync.dma_start(tr31[:], t_tr31[:])
            ident = wp.tile([128, 128], f32, tag="ident")
            make_identity(nc, ident[:])
            iota_i = wp.tile([BQ, T], dt.int32, tag="iotai")
            nc.gpsimd.iota(iota_i[:], pattern=[[1, T]], base=0, channel_multiplier=0)
            iota31f = wp.tile([BQ, T], f32, tag="iota31f")
            nc.vector.tensor_copy(iota31f[:], iota_i[:])
            iotaP_i = wp.tile([T, BQ], dt.int32, tag="iotaPi")
            nc.gpsimd.iota(iotaP_i[:], pattern=[[0, BQ]], base=0, channel_multiplier=1)
            iotaP31f = wp.tile([T, BQ], f32, tag="iotaP31f")
            nc.vector.tensor_copy(iotaP31f[:], iotaP_i[:])
            ones1 = wp.tile([1, T], f32, tag="ones1")
            nc.gpsimd.memset(ones1[:], 1.0)
            ones_row = wp.tile([1, LCH * BQ], f32, tag="ones_row")
            nc.gpsimd.memset(ones_row[:], 1.0)
            toks = []
            for i, (ls, lr) in enumerate(LIN):
                tk = wp.tile([lr, BQ], dt.int32, tag=f"tok{i}", name=f"tok{i}")
                nc.sync.dma_start(tk[:], t_tokT[ls:ls + lr, :])
                toks.append(tk)

            def ceq_view(t):
                return t[:].rearrange("p (e l b) -> p e l b", e=3, l=LCH, b=BQ)

            # ---------- conv phase (e-major output) ----------
            ceq0 = ceqp.tile([128, 3 * LCH * BQ], f32, tag="ceq", name="ceq0")
            with tc.tile_pool(name="embp", bufs=3) as embp, \
                 tc.tile_pool(name="stgp", bufs=3) as stgp:
                for b in range(BQ):
                    embs = []
                    for i, (ls, lr) in enumerate(LIN):
                        et = embp.tile([lr, E + 2], f32, tag=f"emb{i}",
                                       name=f"emb{i}_{b}")
                        nc.gpsimd.memset(et[:, 0:1], 0.0)
                        nc.gpsimd.memset(et[:, E + 1:E + 2], 0.0)
                        nc.gpsimd.indirect_dma_start(
                            out=et[:, 1:1 + E], out_offset=None, in_=t_table[:],
                            in_offset=bass.IndirectOffsetOnAxis(
                                ap=toks[i][:, b:b + 1], axis=0))
                        embs.append(et)
                    for ec, (es, er) in enumerate(KC_CONV):
                        ps = pst.tile([128, max(L, 256)], f32, tag="pt",
                                      name=f"cps{b}_{ec}")
                        first = True
                        for k in range(3):
                            for i, (ls, lr) in enumerate(LIN):
                                nc.tensor.matmul(
                                    ps[0:er, 0:L],
                                    embs[i][:, es + k:es + k + er],
                                    cwT[i][:, k * L:(k + 1) * L],
                                    start=first, stop=(k == 2 and i == len(LIN) - 1))
                                first = False
                        # bias add + scatter into ring chunk 0 / spill
                        nc.vector.tensor_tensor(
                            out=ceq_view(ceq0)[0:er, ec, :, b],
                            in0=ps[0:er, 0:LCH], in1=cb_rep[0:er, 0:LCH],
                            op=OP.add)
                        if NCH > 1:
                            stg = stgp.tile([128, L - LCH], f32, tag="stg",
                                            name=f"stg{b}_{ec}")
                            nc.vector.tensor_tensor(
                                out=stg[0:er, :], in0=ps[0:er, LCH:L],
                                in1=cb_rep[0:er, LCH:L], op=OP.add)
                            for ch in range(1, NCH):
                                nc.sync.dma_start(
                                    d_spill[ch - 1, b, ec, 0:er, :],
                                    stg[0:er, (ch - 1) * LCH:ch * LCH])
            nc.sync.dma_start(
                ceq0[EA - 256 - 1:EA - 256, 2 * LCH * BQ:3 * LCH * BQ], ones_row[:])

            # ---------- recurrence ----------
            with tc.tile_pool(name="gps", bufs=1, space="PSUM") as gps, \
                 tc.tile_pool(name="emp", bufs=1, space="PSUM") as emp, \
                 tc.tile_pool(name="sw", bufs=2) as sw, \
                 tc.tile_pool(name="cp", bufs=2) as cp, \
                 tc.tile_pool(name="hp", bufs=2) as hp, \
                 tc.tile_pool(name="hTp", bufs=2) as hTp:

                c_prev = cp.tile([BQ, H], f32, tag="c", name="cinit")
                nc.gpsimd.memset(c_prev[:], 0.0)
                hT_tiles = None
                cur = ceq0
                for s in range(L):
                    ch, li = s // LCH, s % LCH
                    if li == 0:
                        if ch > 0:
                            cur = nxt
                        if ch + 1 < NCH:  # prefetch next chunk
                            nxt = ceqp.tile([128, 3 * LCH * BQ], f32, tag="ceq",
                                            name=f"ceq{ch + 1}")
                            for b in range(BQ):
                                for ec, (es, er) in enumerate(KC_CONV):
                                    nc.sync.dma_start(
                                        ceq_view(nxt)[0:er, ec, :, b],
                                        d_spill[ch, b, ec, 0:er, :])
                            nc.sync.dma_start(
                                nxt[EA - 256 - 1:EA - 256,
                                    2 * LCH * BQ:3 * LCH * BQ], ones_row[:])
                    cv = ceq_view(cur)
                    gates = []
                    for jb in range(4):
                        g = gps.tile([BQ, 512], f32, tag=f"g{jb}",
                                     name=f"g{s}_{jb}")
                        for kc, (es, er) in enumerate(KC_E):
                            nc.tensor.matmul(
                                g[:], cv[0:er, kc, li, :],
                                wih[0:er, kc * G + jb * 512:kc * G + (jb + 1) * 512],
                                start=(kc == 0), stop=(s == 0 and kc == 2))
                        if s > 0:
                            for kc in range(NKH):
                                nc.tensor.matmul(
                                    g[:], hT_tiles[kc][:],
                                    whh[:, kc * G + jb * 512:kc * G + (jb + 1) * 512],
                                    start=False, stop=(kc == NKH - 1))
                        gates.append(g)
                    si = sw.tile([BQ, 512], f32, tag="si", name=f"si{s}")
                    nc.scalar.activation(si[:], gates[0][:], AF.Sigmoid)
                    sf = sw.tile([BQ, 512], f32, tag="sf", name=f"sf{s}")
                    nc.scalar.activation(sf[:], gates[1][:], AF.Sigmoid)
                    tg = sw.tile([BQ, 512], f32, tag="tg", name=f"tg{s}")
                    nc.scalar.activation(tg[:], gates[2][:], AF.Tanh)
                    so = sw.tile([BQ, 512], f32, tag="so", name=f"so{s}")
                    nc.scalar.activation(so[:], gates[3][:], AF.Sigmoid)
                    m1 = sw.tile([BQ, 512], f32, tag="m1", bufs=1, name=f"m1{s}")
                    nc.vector.tensor_tensor(out=m1[:], in0=si[:], in1=tg[:],
                                            op=OP.mult)
                    m2 = sw.tile([BQ, 512], f32, tag="m2", bufs=1, name=f"m2{s}")
                    nc.vector.tensor_tensor(out=m2[:], in0=sf[:], in1=c_prev[:],
                                            op=OP.mult)
                    c_new = cp.tile([BQ, H], f32, tag="c", name=f"c{s}")
                    nc.vector.tensor_tensor(out=c_new[:], in0=m1[:], in1=m2[:],
                                            op=OP.add)
                    tcn = sw.tile([BQ, 512], f32, tag="tcn", bufs=1, name=f"tcn{s}")
                    nc.scalar.activation(tcn[:], c_new[:], AF.Tanh)
                    h = hp.tile([BQ, H], f32, tag="h", name=f"h{s}")
                    nc.vector.tensor_tensor(out=h[:], in0=so[:], in1=tcn[:],
                                            op=OP.mult)
                    c_prev = c_new
                    pt = pst.tile([128, max(L, 256)], f32, tag="pt", name=f"pt{s}")
                    for k in range(NKH):
                        nc.tensor.transpose(pt[:, k * BQ:(k + 1) * BQ],
                                            h[:, k * 128:(k + 1) * 128],
                                            ident[0:BQ, 0:BQ])
                    newhT = []
                    for k in range(NKH):
                        t = hTp.tile([128, BQ], f32, tag=f"hT{k}",
                                     name=f"hT{k}_{s}")
                        nc.vector.tensor_copy(t[:], pt[:, k * BQ:(k + 1) * BQ])
                        newhT.append(t)
                    hT_tiles = newhT
                    # inline classifier partials (A = canonical slot s for fwd
                    # cores, B = canonical slot L-1-s for bwd cores)
                    ea = emp.tile([BQ, T], f32, tag="ea", name=f"ea{s}")
                    eb = emp.tile([BQ, T], f32, tag="eb", name=f"eb{s}")
                    for kc in range(NKH):
                        nc.tensor.matmul(ea[:], hT_tiles[kc][:],
                                         cls[:, kc * T:(kc + 1) * T],
                                         start=(kc == 0), stop=(kc == NKH - 1))
                    for kc in range(NKH):
                        nc.tensor.matmul(eb[:], hT_tiles[kc][:],
                                         cls[:, (NKH + kc) * T:(NKH + kc + 1) * T],
                                         start=(kc == 0), stop=(kc == NKH - 1))
                    sA, sB = s, L - 1 - s
                    if s <= L // 2 - 1:  # first writer of both slots
                        nc.scalar.copy(em_sb[:, sA * T:(sA + 1) * T], ea[:])
                        nc.vector.tensor_copy(em_sb[:, sB * T:(sB + 1) * T], eb[:])
                    else:
                        nc.vector.tensor_tensor(
                            out=em_sb[:, sA * T:(sA + 1) * T],
                            in0=em_sb[:, sA * T:(sA + 1) * T], in1=ea[:], op=OP.add)
                        nc.vector.tensor_tensor(
                            out=em_sb[:, sB * T:(sB + 1) * T],
                            in0=em_sb[:, sB * T:(sB + 1) * T], in1=eb[:], op=OP.add)

            # ---------- pair AllReduce of emission partials ----------
            nc.sync.dma_start(d_ccin.ap(), em_sb[:, :])
            nc.gpsimd.collective_compute(
                "AllReduce", mybir.AluOpType.add,
                replica_groups=[[0, 1], [2, 3], [4, 5], [6, 7]],
                ins=[d_ccin.ap()], outs=[d_ccout.ap()])
            nc.sync.dma_start(em_sb[:, :], d_ccout.ap())
            em3 = em_sb.rearrange("p (l t) -> p l t", t=T)
            nc.vector.tensor_tensor(
                out=em3, in0=em3,
                in1=clsb_rep[:].unsqueeze(1).broadcast_to([BQ, L, T]), op=OP.add)

            # ---------- viterbi forward ----------
            with tc.tile_pool(name="vtp", bufs=2) as vtp, \
                 tc.tile_pool(name="vbp", bufs=2) as vbp, \
                 tc.tile_pool(name="btp", bufs=2, space="PSUM") as btp:
                nc.vector.tensor_tensor(out=sc_sb[:, 0:T], in0=em_sb[:, 0:T],
                                        in1=st_rep[:], op=OP.add)
                for l in range(1, L):
                    vt = vtp.tile([BQ, T * T], f32, tag="vt", name=f"vt{l}")
                    nc.vector.tensor_tensor(
                        out=vt[:].rearrange("p (j i) -> p j i", i=T),
                        in0=sc_sb[:, (l - 1) * T:l * T].unsqueeze(1)
                            .broadcast_to([BQ, T, T]),
                        in1=trT_rep[:].rearrange("p (j i) -> p j i", i=T),
                        op=OP.add)
                    vb = vbp.tile([BQ, T], f32, tag="vb", name=f"vb{l}")
                    nc.vector.tensor_reduce(
                        out=vb[:], in_=vt[:].rearrange("p (j i) -> p j i", i=T),
                        axis=mybir.AxisListType.X, op=OP.max)
                    nc.vector.tensor_tensor(
                        out=sc_sb[:, l * T:(l + 1) * T], in0=vb[:],
                        in1=em_sb[:, l * T:(l + 1) * T], op=OP.add)
                # final + argmax
                fin = vbp.tile([BQ, T], f32, tag="fin")
                nc.vector.tensor_tensor(out=fin[:], in0=sc_sb[:, (L - 1) * T:L * T],
                                        in1=et_rep[:], op=OP.add)
                bs = vbp.tile([BQ, 1], f32, tag="bs")
                nc.vector.tensor_reduce(out=bs[:], in_=fin[:],
                                        axis=mybir.AxisListType.X, op=OP.max)
                nc.sync.dma_start(o_score.ap(), bs[:])

                def argmax_into(vec, mx, dst, uid):
                    eq = vbp.tile([BQ, T], f32, tag="eq", name=f"eq{uid}")
                    nc.vector.tensor_scalar(out=eq[:], in0=vec, scalar1=mx,
                                            scalar2=None, op0=OP.is_equal)
                    eqb = vbp.tile([BQ, T], f32, tag="eqb", name=f"eqb{uid}")
                    nc.vector.tensor_scalar(out=eqb[:], in0=eq[:], scalar1=-1.0e4,
                                            scalar2=1.0e4, op0=OP.mult, op1=OP.add)
                    cand = vbp.tile([BQ, T], f32, tag="cand", name=f"cand{uid}")
                    nc.vector.tensor_tensor(out=cand[:], in0=iota31f[:],
                                            in1=eqb[:], op=OP.add)
                    nc.vector.tensor_reduce(out=dst, in_=cand[:],
                                            axis=mybir.AxisListType.X, op=OP.min)

                argmax_into(fin[:], bs[:, 0:1], tags_sb[:, L - 1:L], "fin")

                # ---------- backtrace ----------
                for l in range(L - 1, 0, -1):
                    p1 = btp.tile([1, BQ], f32, tag="bt1", name=f"bt1_{l}")
                    nc.tensor.matmul(p1[:], tags_sb[:, l:l + 1],
                                     ident[0:BQ, 0:BQ], start=True, stop=True)
                    tT = vbp.tile([1, BQ], f32, tag="tT", name=f"tT{l}")
                    nc.scalar.copy(tT[:], p1[:])
                    p2 = btp.tile([T, BQ], f32, tag="bt2", name=f"bt2_{l}")
                    nc.tensor.matmul(p2[:], ones1[:], tT[:], start=True, stop=True)
                    oh = vbp.tile([T, BQ], f32, tag="oh", name=f"oh{l}")
                    nc.vector.tensor_tensor(out=oh[:], in0=iotaP31f[:], in1=p2[:],
                                            op=OP.is_equal)
                    p3 = btp.tile([BQ, T], f32, tag="bt3", name=f"bt3_{l}")
                    nc.tensor.matmul(p3[:], oh[:], tr31[:], start=True, stop=True)
                    v = vbp.tile([BQ, T], f32, tag="v", name=f"v{l}")
                    nc.vector.tensor_tensor(out=v[:], in0=p3[:],
                                            in1=sc_sb[:, (l - 1) * T:l * T],
                                            op=OP.add)
                    m = vbp.tile([BQ, 1], f32, tag="m", name=f"m{l}")
                    nc.vector.tensor_reduce(out=m[:], in_=v[:],
                                            axis=mybir.AxisListType.X, op=OP.max)
                    argmax_into(v[:], m[:, 0:1], tags_sb[:, l - 1:l], l)

                ti = vbp.tile([BQ, L], dt.int32, tag="ti")
                nc.vector.tensor_copy(ti[:], tags_sb[:, :])
                nc.sync.dma_start(o_tags.ap(), ti[:])

    nc.compile()
    return nc


def _prep_core_inputs(c, tokens, table, conv_w, conv_b, w_ih_f, w_hh_f, b_f,
                      w_ih_b, w_hh_b, b_b, cls_w, cls_b, start_t, end_t, trans):
    B, L = tokens.shape
    BQ = B // 4
    q, isb = c // 2, c % 2
    f32 = np.float32
    tk = tokens[q * BQ:(q + 1) * BQ]
    if isb:
        tk = tk[:, ::-1]
        cw = conv_w[::-1, ::-1, :]
        cb = conv_b[::-1]
        w_ih, w_hh, bias = w_ih_b, w_hh_b, b_b
        cls_half = cls_w[:, H:2 * H]
    else:
        cw = conv_w
        cb = conv_b
        w_ih, w_hh, bias = w_ih_f, w_hh_f, b_f
        cls_half = cls_w[:, 0:H]
    cwT = np.ascontiguousarray(np.transpose(cw, (1, 2, 0)).reshape(L, 3 * L))
    wihT = np.concatenate([w_ih.T, bias[None, :]], axis=0)  # [301, G]
    wih_pad = np.zeros((3, 128, G), f32)
    for kc, (es, er) in enumerate([(0, 128), (128, 128), (256, EA - 256)]):
        wih_pad[kc, 0:er] = wihT[es:es + er]
    whhT = w_hh.T  # [512, G]
    whh_pad = np.zeros((H // 128, 128, G), f32)
    for kc in range(H // 128):
        whh_pad[kc] = whhT[kc * 128:(kc + 1) * 128]
    clsT = cls_half.T  # [512, T]
    cls_pad = np.zeros((2, H // 128, 128, T), f32)
    for kc in range(H // 128):
        cls_pad[isb, kc] = clsT[kc * 128:(kc + 1) * 128]
    return {
        "tokT": np.ascontiguousarray(tk.T.astype(np.int32)),
        "table": table,
        "cwT": cwT.astype(f32),
        "cb": np.ascontiguousarray(cb.astype(f32)[None, :]),
        "wih": wih_pad,
        "whh": whh_pad,
        "cls": cls_pad,
        "clsb": cls_b.astype(f32)[None, :],
        "st": start_t.astype(f32)[None, :],
        "et": end_t.astype(f32)[None, :],
        "trT": np.ascontiguousarray(trans.T.astype(f32).reshape(1, T * T)),
        "tr31": np.ascontiguousarray(trans.T.astype(f32)),
    }


def kernel(**inputs):
    from concourse.bass_utils import run_bass_kernel_spmd

    inp = {k: np.asarray(v) for k, v in inputs.items()}
    tokens = inp["inputs"].astype(np.int32)
    B, L = tokens.shape
    BQ = B // 4
    key = (L, BQ)
    if key not in _CACHE:
        _CACHE[key] = _build(L, BQ)
    nc = _CACHE[key]

    args = (tokens, inp["embed_table"].astype(np.float32),
            inp["conv_w"].astype(np.float32), inp["conv_b"].astype(np.float32),
            inp["w_ih_f"].astype(np.float32), inp["w_hh_f"].astype(np.float32),
            inp["b_f"].astype(np.float32),
            inp["w_ih_b"].astype(np.float32), inp["w_hh_b"].astype(np.float32),
            inp["b_b"].astype(np.float32),
            inp["cls_w"].astype(np.float32), inp["cls_b"].astype(np.float32),
            inp["start_t"].astype(np.float32), inp["end_t"].astype(np.float32),
            inp["trans"].astype(np.float32))
    in_maps = [_prep_core_inputs(c, *args) for c in range(8)]
    res = run_bass_kernel_spmd(nc, in_maps, core_ids=list(range(8)),
                               trace=bool(globals().get("TRACE")))
    globals()["LAST_EXEC_NS"] = res.exec_time_ns
    tags = np.zeros((B, L), np.int32)
    score = np.zeros((B,), np.float32)
    for q in range(4):
        r = res.results[2 * q]
        tags[q * BQ:(q + 1) * BQ] = r["o_tags"]
        score[q * BQ:(q + 1) * BQ] = r["o_score"][:, 0]
    return tags, score
